# revision 2
# baseline (speedup 1.0000x reference)
"""DynamicConv1dTBC Trainium2 Bass kernel, v2.

Problem: x [T=2048, B=4, C=1024] f32, Wlin [240, 1024] f32.
  w = softmax(einsum('tbc,kc->tbk', x, Wlin).reshape(T,B,H=16,K=15), axis=-1)
  out[t,b,h,r] = sum_k w[t,b,h,k] * xpad[t+k, b, h*64+r]   (causal, PAD_L=14)

Sharding: T split across 8 cores (256 out-timesteps each + 14-row left halo).

v2 band construction: the shear (t,k)->(s=t+k, t) is LINEAR in t, so the
softmaxed weights are written to DRAM with a diagonal row stride
32*(SW+1) while (k,h,oc) stays contiguous (960B runs), then read back as
A_pair[t, (s,h,oc)] with row pitch 32*SW (9088B runs). One write + one
read per batch b replaces zero-padded pitch-trick staging. Weight-gen
emits k-major logits via a column-permuted WlinT so the staging source
is contiguous. The b-loop is software-pipelined: band transposes + conv
of batch b-1 are emitted between staging(b) and readback(b) so the PE
queue never stalls on the DRAM round trip.
"""
import sys, os
for _p in ("/opt/trn_rl_repo",):
    if _p not in sys.path and os.path.isdir(_p):
        sys.path.insert(0, _p)

import numpy as np
from contextlib import ExitStack

import concourse.bass as bass
import concourse.tile as tile
from concourse import mybir, bacc, masks
from concourse._compat import with_exitstack
from concourse.bass_utils import run_bass_kernel_spmd

# ---- problem constants -------------------------------------------------------
T_GLOBAL, B, C = 2048, 4, 1024
H, K, R = 16, 15, 64
J = H * K                      # 240
PAD_L = K - 1                  # 14
N_CORES = 8
T_LOC = T_GLOBAL // N_CORES    # 256 output timesteps per core
T_EXT = T_LOC + PAD_L          # 270 input rows per core
OC = 2                         # out-chunks of 128 timesteps
TCH = T_LOC // OC              # 128
SW = TCH + PAD_L               # 142 s-window per chunk
GW = H * OC                    # 32 elems per s position (h,oc interleaved)
ROW_RD = SW * GW               # 4544 elems per t row (read pitch)
ROW_WR = (SW + 1) * GW         # 4576 elems (write pitch = diagonal shear)
F32 = mybir.dt.float32
BF16 = mybir.dt.bfloat16


def make_adram(pool):
    return {b: pool.tile([TCH, ROW_RD], BF16, name=f"ad{b}", tag=f"ad{b}")
            for b in range(B)}


@with_exitstack
def dynconv_prelude(ctx: ExitStack, tc: tile.TileContext, adram: dict,
                    ident, ident_f):
    """One-time setup: transpose identities + zero the diagonal-staging DRAM
    buffers (steady-state writes only touch the same (t,k) stripes, so the
    zeros elsewhere persist across reps)."""
    nc = tc.nc
    masks.make_identity(nc, ident[:])
    masks.make_identity(nc, ident_f[:])
    zp = ctx.enter_context(tc.tile_pool(name="zfill", bufs=1))
    zsrc = zp.tile([TCH, ROW_RD], BF16)
    nc.gpsimd.memset(zsrc[:], 0.0)
    for b in range(B):
        at = adram[b][:]
        dst = bass.AP(at.tensor, at.offset, [[ROW_RD, TCH], [1, ROW_RD]])
        nc.sync.dma_start(dst, zsrc[:])


@with_exitstack
def dynconv_kernel(ctx: ExitStack, tc: tile.TileContext,
                   x_ap: bass.AP, wlin_ap: bass.AP, out_ap: bass.AP,
                   adram: dict, ident, ident_f):
    nc = tc.nc
    xpool = ctx.enter_context(tc.tile_pool(name="x", bufs=2))
    wl = ctx.enter_context(tc.tile_pool(name="wl", bufs=2))
    xtp = ctx.enter_context(tc.tile_pool(name="xt", bufs=2))
    wex = ctx.enter_context(tc.tile_pool(name="wex", bufs=2))
    wnp = ctx.enter_context(tc.tile_pool(name="wn", bufs=2))
    anat = ctx.enter_context(tc.tile_pool(name="anat", bufs=2))
    bandp = ctx.enter_context(tc.tile_pool(name="band", bufs=3))
    stg = ctx.enter_context(tc.tile_pool(name="stg", bufs=3))
    ps_w = ctx.enter_context(tc.tile_pool(name="psw", bufs=2, space="PSUM"))
    ps_t = ctx.enter_context(tc.tile_pool(name="pst", bufs=3, space="PSUM"))
    ps_a = ctx.enter_context(tc.tile_pool(name="psa", bufs=1, space="PSUM"))
    ps_c = ctx.enter_context(tc.tile_pool(name="psc", bufs=2, space="PSUM"))

    # ---- Wlin load (f32 on sync HWDGE) + WlinT build -------------------------
    # column-permuted: j' = k*16 + h (k-major) so weight-gen output is
    # directly stageable with (k h) contiguous; f32->bf16 cast happens on
    # the PSUM-evac copy after the (f32-input) transpose.
    wlin_b = wl.tile([120, C], F32)
    nc.sync.dma_start(wlin_b[:], wlin_ap[0:120, :])
    wlin_b2 = wl.tile([120, C], F32)
    nc.sync.dma_start(wlin_b2[:], wlin_ap[120:240, :])

    # ---- x loads (cast f32 -> bf16 during DMA on gpsimd) ---------------------
    xv = x_ap.rearrange("t b c -> t (b c)")
    x_oc = [xpool.tile([TCH, B * C], BF16, name=f"xoc{i}", tag=f"xoc{i}")
            for i in range(OC)]
    x_pre = xpool.tile([PAD_L, B * C], BF16, tag="xpre")
    x_mid = xpool.tile([PAD_L, B * C], BF16, tag="xmid")
    nc.gpsimd.dma_start(x_oc[0][:], xv[PAD_L:PAD_L + TCH, :])
    nc.gpsimd.dma_start(x_oc[1][:], xv[PAD_L + TCH:T_EXT, :])
    nc.gpsimd.dma_start(x_pre[:], xv[0:PAD_L, :])
    nc.gpsimd.dma_start(x_mid[:], xv[TCH:TCH + PAD_L, :])

    wlinT = []
    for cc in range(8):
        wt = wl.tile([128, J], BF16, name=f"wlinT{cc}", tag=f"wlinT{cc}")
        for i, wb in enumerate((wlin_b, wlin_b2)):
            pt = ps_t.tile([128, 120], F32, tag="ptt")
            nc.tensor.transpose(pt[:], wb[:, cc * 128:(cc + 1) * 128],
                                ident_f[:])
            # scatter (h k) -> columns k*16 + h (h in [8i, 8i+8))
            wtv, ptv = wt[:], pt[:]
            dst = bass.AP(wtv.tensor, wtv.offset + 8 * i,
                          [[wtv.ap[0][0], 128], [1, 8], [16, 15]])
            src = bass.AP(ptv.tensor, ptv.offset,
                          [[ptv.ap[0][0], 128], [15, 8], [1, 15]])
            nc.vector.tensor_copy(dst, src)
        wlinT.append(wt)

    def emit_phase1(b):
        """weight-gen + softmax for both oc; returns wn_pair tile."""
        wn_pair = wnp.tile([TCH, GW * K], BF16)     # [t, (k h oc)]
        for oc in range(OC):
            cs = b * C
            pw = ps_w.tile([TCH, J], F32)
            ptx = ps_t.tile([128, 8 * TCH], BF16, tag="ptt")
            for cc in range(8):
                nc.tensor.matmul(ptx[:, cc * TCH:(cc + 1) * TCH],
                                 x_oc[oc][:, cs + cc * 128:cs + (cc + 1) * 128],
                                 ident[:], is_transpose=True,
                                 skip_group_check=True)
            xTw = xtp.tile([128, 8 * TCH], BF16, tag="xTw")
            nc.vector.tensor_copy(xTw[:], ptx[:])
            for cc in range(8):
                nc.tensor.matmul(pw[:], xTw[:, cc * TCH:(cc + 1) * TCH],
                                 wlinT[cc][:], start=(cc == 0), stop=(cc == 7))
            # softmax over k (pw is [t, (k h)]: k stride 16, h stride 1)
            we = wex.tile([TCH, J], F32)
            nc.scalar.activation(we[:], pw[:], mybir.ActivationFunctionType.Exp)
            sums = wex.tile([TCH, H], F32, tag="sums")
            nc.vector.reduce_sum(sums[:],
                                 we[:].rearrange("t (k h) -> t h k", h=H),
                                 axis=mybir.AxisListType.X)
            inv = wex.tile([TCH, H], F32, tag="invs")
            nc.vector.reciprocal(inv[:], sums[:])
            # normalize into wn_pair[t, k*32 + h*2 + oc] (iterate t, k, h)
            wnv, wev = wn_pair[:], we[:]
            dst = bass.AP(wnv.tensor, wnv.offset + oc,
                          [[wnv.ap[0][0], TCH], [GW, K], [2, H]])
            src = bass.AP(wev.tensor, wev.offset,
                          [[wev.ap[0][0], TCH], [H, K], [1, H]])
            nc.vector.tensor_tensor(
                dst, src, inv[:].unsqueeze(1).broadcast_to((TCH, K, H)),
                op=mybir.AluOpType.mult)
        return wn_pair

    def emit_stage_write(b, wn_pair):
        at = adram[b][:]
        dstw = bass.AP(at.tensor, at.offset, [[ROW_WR, TCH], [1, GW * K]])
        nc.sync.dma_start(dstw, wn_pair[:])

    def emit_readback(b):
        at = adram[b][:]
        apair = anat.tile([TCH, ROW_RD], BF16)
        srcr = bass.AP(at.tensor, at.offset, [[ROW_RD, TCH], [1, ROW_RD]])
        nc.sync.dma_start(apair[:], srcr)
        return apair

    def emit_phase3(b, apair):
        """band transposes + conv + evac + store for both oc of batch b."""
        apv = apair[:]
        p0 = apv.ap[0][0]
        for oc in range(OC):
            # halo bands: B[s<14, t] is nonzero only for t <= s < 14, so
            # transpose just the [14 t, 14 s] corner per h, packed as column
            # slices of one psum tile, single evac.
            pah = ps_a.tile([PAD_L, H * PAD_L], BF16, tag="pta")
            for h in range(H):
                src = bass.AP(apv.tensor, apv.offset + h * 2 + oc,
                              [[p0, PAD_L], [GW, PAD_L]])
                nc.tensor.matmul(pah[:, h * PAD_L:(h + 1) * PAD_L],
                                 src, ident[0:PAD_L, 0:PAD_L],
                                 is_transpose=True, skip_group_check=True)
            ba_c = bandp.tile([PAD_L, H * PAD_L], BF16, tag="ba")
            nc.vector.tensor_copy(ba_c[:], pah[:])
            stage = stg.tile([TCH, C], F32)
            for hq in range(2):
                # main bands: 8 transposes [128 t, 128 s] -> [128 s, 128 t]
                pbq = ps_t.tile([128, 8 * TCH], BF16, tag="ptt")
                for j in range(8):
                    h = hq * 8 + j
                    src = bass.AP(apv.tensor,
                                  apv.offset + PAD_L * GW + h * 2 + oc,
                                  [[p0, TCH], [GW, TCH]])
                    nc.tensor.matmul(pbq[:, j * TCH:(j + 1) * TCH],
                                     src, ident[:], is_transpose=True,
                                     skip_group_check=True)
                bwq = bandp.tile([128, 8 * TCH], BF16, tag="bw")
                if hq == 0:
                    nc.vector.tensor_copy(bwq[:], pbq[:])
                else:
                    nc.scalar.activation(bwq[:], pbq[:],
                                         mybir.ActivationFunctionType.Copy)
                # conv: out[t, r] = sum_s band[s, t] * x[s, b, h*64+r]
                pc = ps_c.tile([TCH, 8 * R], F32)
                xh_t = x_pre if oc == 0 else x_mid
                for j in range(8):
                    h = hq * 8 + j
                    fo = b * C + h * R
                    nc.tensor.matmul(pc[:, j * R:(j + 1) * R],
                                     bwq[:, j * TCH:(j + 1) * TCH],
                                     x_oc[oc][:, fo:fo + R],
                                     start=True, stop=False,
                                     skip_group_check=True)
                    nc.tensor.matmul(pc[0:PAD_L, j * R:(j + 1) * R],
                                     ba_c[:, h * PAD_L:(h + 1) * PAD_L],
                                     xh_t[:, fo:fo + R],
                                     start=False, stop=True,
                                     skip_group_check=True)
                if hq == 0:
                    nc.vector.tensor_copy(
                        stage[:, hq * 8 * R:(hq + 1) * 8 * R], pc[:])
                else:
                    nc.scalar.activation(
                        stage[:, hq * 8 * R:(hq + 1) * 8 * R], pc[:],
                        mybir.ActivationFunctionType.Copy)
            nc.sync.dma_start(out_ap[oc * TCH:(oc + 1) * TCH, b, :], stage[:])

    # ---- software-pipelined b loop: phase3 lags by one batch -----------------
    apairs = {}
    for b in range(B):
        wn = emit_phase1(b)
        emit_stage_write(b, wn)
        if b > 0:
            emit_phase3(b - 1, apairs.pop(b - 1))
        apairs[b] = emit_readback(b)
    emit_phase3(B - 1, apairs.pop(B - 1))


def build_program(debug=False, reps=1):
    nc = bacc.Bacc("TRN2", target_bir_lowering=False, debug=debug,
                   enable_asserts=False, num_devices=N_CORES)
    x_t = nc.dram_tensor("x", [T_EXT, B, C], F32, kind="ExternalInput")
    wlin_t = nc.dram_tensor("wlin", [J, C], F32, kind="ExternalInput")
    out_t = nc.dram_tensor("out", [T_LOC, B, C], F32, kind="ExternalOutput")
    with tile.TileContext(nc) as tc:
        with tc.tile_pool(name="addram", bufs=1, space="DRAM") as apool, \
             tc.tile_pool(name="const", bufs=1) as const:
            adram = make_adram(apool)
            ident = const.tile([128, 128], BF16)
            ident_f = const.tile([120, 120], F32)
            if os.environ.get("NOPRELUDE") != "1":
                dynconv_prelude(tc, adram, ident, ident_f)
            if reps == 1:
                dynconv_kernel(tc, x_t.ap(), wlin_t.ap(), out_t.ap(),
                               adram, ident, ident_f)
            else:
                with tc.For_i(0, reps, 1):
                    dynconv_kernel(tc, x_t.ap(), wlin_t.ap(), out_t.ap(),
                                   adram, ident, ident_f)
    nc.compile()
    return nc


_NC_CACHE = None


def kernel(x: np.ndarray, Wlin: np.ndarray) -> np.ndarray:
    global _NC_CACHE
    if _NC_CACHE is None:
        _NC_CACHE = build_program()
    nc = _NC_CACHE
    xp = np.pad(x, ((PAD_L, 0), (0, 0), (0, 0)))
    in_maps = []
    for i in range(N_CORES):
        in_maps.append({
            "x": np.ascontiguousarray(xp[i * T_LOC:i * T_LOC + T_EXT]),
            "wlin": np.ascontiguousarray(Wlin),
        })
    res = run_bass_kernel_spmd(nc, in_maps, core_ids=list(range(N_CORES)))
    outs = [res.results[i]["out"] for i in range(N_CORES)]
    return np.concatenate(outs, axis=0)


# revision 3
# speedup vs baseline: 5.4708x; 5.4708x over previous
"""DynamicConv1dTBC Trainium2 Bass kernel, v2.

Problem: x [T=2048, B=4, C=1024] f32, Wlin [240, 1024] f32.
  w = softmax(einsum('tbc,kc->tbk', x, Wlin).reshape(T,B,H=16,K=15), axis=-1)
  out[t,b,h,r] = sum_k w[t,b,h,k] * xpad[t+k, b, h*64+r]   (causal, PAD_L=14)

Sharding: T split across 8 cores (256 out-timesteps each + 14-row left halo).

v2 band construction: the shear (t,k)->(s=t+k, t) is LINEAR in t, so the
softmaxed weights are written to DRAM with a diagonal row stride
16*(SW+1) while (k,h) stays contiguous, then read back as
A[t, (s,h)] with row pitch 16*SW (4544B runs). One write + one read per
(b, oc) unit replaces the zero-padded pitch-trick staging. Weight-gen
emits k-major logits via a column-permuted WlinT so the staging source
is contiguous. The (b, oc) unit loop is software-pipelined with skew 2:
band transposes + conv of unit u-2 are emitted between staging(u) and
readback(u) so the in-order PE queue never stalls on the DRAM round trip.
"""
import sys, os
for _p in ("/opt/trn_rl_repo",):
    if _p not in sys.path and os.path.isdir(_p):
        sys.path.insert(0, _p)

import numpy as np
from contextlib import ExitStack

import concourse.bass as bass
import concourse.tile as tile
from concourse import mybir, bacc, masks
from concourse._compat import with_exitstack
from concourse.bass_utils import run_bass_kernel_spmd

# ---- problem constants -------------------------------------------------------
T_GLOBAL, B, C = 2048, 4, 1024
H, K, R = 16, 15, 64
J = H * K                      # 240
PAD_L = K - 1                  # 14
N_CORES = 8
T_LOC = T_GLOBAL // N_CORES    # 256 output timesteps per core
T_EXT = T_LOC + PAD_L          # 270 input rows per core
OC = 2                         # out-chunks of 128 timesteps
TCH = T_LOC // OC              # 128
SW = TCH + PAD_L               # 142 s-window per chunk
ROW_RD = SW * H                # 2272 elems per t row (read pitch)
ROW_WR = (SW + 1) * H          # 2288 elems (write pitch = diagonal shear)
SKEW = 2                       # software pipeline depth (units)
F32 = mybir.dt.float32
BF16 = mybir.dt.bfloat16


def make_adram(pool):
    return {(b, oc): pool.tile([TCH, ROW_RD], BF16, name=f"ad{b}_{oc}",
                               tag=f"ad{b}_{oc}")
            for b in range(B) for oc in range(OC)}


@with_exitstack
def dynconv_prelude(ctx: ExitStack, tc: tile.TileContext, adram: dict,
                    ident, ident_f):
    """One-time setup: transpose identities + zero the diagonal-staging DRAM
    buffers (steady-state writes only touch the same (t,k) stripes, so the
    zeros elsewhere persist across reps)."""
    nc = tc.nc
    masks.make_identity(nc, ident[:])
    masks.make_identity(nc, ident_f[:])
    zp = ctx.enter_context(tc.tile_pool(name="zfill", bufs=1))
    zsrc = zp.tile([TCH, ROW_RD], BF16)
    nc.gpsimd.memset(zsrc[:], 0.0)
    for u, at in adram.items():
        atv = at[:]
        dst = bass.AP(atv.tensor, atv.offset, [[ROW_RD, TCH], [1, ROW_RD]])
        nc.sync.dma_start(dst, zsrc[:])


@with_exitstack
def dynconv_kernel(ctx: ExitStack, tc: tile.TileContext,
                   x_ap: bass.AP, wlin_ap: bass.AP, out_ap: bass.AP,
                   adram: dict, ident, ident_f):
    nc = tc.nc
    xpool = ctx.enter_context(tc.tile_pool(name="x", bufs=2))
    wl = ctx.enter_context(tc.tile_pool(name="wl", bufs=2))
    xtp = ctx.enter_context(tc.tile_pool(name="xt", bufs=2))
    wex = ctx.enter_context(tc.tile_pool(name="wex", bufs=2))
    wnp = ctx.enter_context(tc.tile_pool(name="wn", bufs=2))
    anat = ctx.enter_context(tc.tile_pool(name="anat", bufs=3))
    bandp = ctx.enter_context(tc.tile_pool(name="band", bufs=3))
    stg = ctx.enter_context(tc.tile_pool(name="stg", bufs=3))
    ps_w = ctx.enter_context(tc.tile_pool(name="psw", bufs=2, space="PSUM"))
    ps_t = ctx.enter_context(tc.tile_pool(name="pst", bufs=3, space="PSUM"))
    ps_a = ctx.enter_context(tc.tile_pool(name="psa", bufs=1, space="PSUM"))
    ps_c = ctx.enter_context(tc.tile_pool(name="psc", bufs=2, space="PSUM"))

    # ---- Wlin load (f32 on sync HWDGE) + x loads (bf16 cast on gpsimd) ------
    wlin_b = wl.tile([120, C], F32)
    nc.sync.dma_start(wlin_b[:], wlin_ap[0:120, :])
    wlin_b2 = wl.tile([120, C], F32)
    nc.sync.dma_start(wlin_b2[:], wlin_ap[120:240, :])

    xv = x_ap.rearrange("t b c -> t (b c)")
    x_oc = [xpool.tile([TCH, B * C], BF16, name=f"xoc{i}", tag=f"xoc{i}")
            for i in range(OC)]
    x_pre = xpool.tile([PAD_L, B * C], BF16, tag="xpre")
    x_mid = xpool.tile([PAD_L, B * C], BF16, tag="xmid")
    nc.gpsimd.dma_start(x_oc[0][:], xv[PAD_L:PAD_L + TCH, :])
    nc.gpsimd.dma_start(x_oc[1][:], xv[PAD_L + TCH:T_EXT, :])
    nc.gpsimd.dma_start(x_pre[:], xv[0:PAD_L, :])
    nc.gpsimd.dma_start(x_mid[:], xv[TCH:TCH + PAD_L, :])

    # ---- WlinT build: [128 c, 240 j'] bf16, j' = k*16 + h (k-major) ---------
    # f32->bf16 cast happens on the PSUM-evac copy after the f32 transpose.
    wlinT = []
    for cc in range(8):
        wt = wl.tile([128, J], BF16, name=f"wlinT{cc}", tag=f"wlinT{cc}")
        for i, wb in enumerate((wlin_b, wlin_b2)):
            pt = ps_t.tile([128, 120], F32, tag="ptt")
            nc.tensor.transpose(pt[:], wb[:, cc * 128:(cc + 1) * 128],
                                ident_f[:])
            # scatter (h k) -> columns k*16 + h (h in [8i, 8i+8))
            wtv, ptv = wt[:], pt[:]
            dst = bass.AP(wtv.tensor, wtv.offset + 8 * i,
                          [[wtv.ap[0][0], 128], [1, 8], [16, 15]])
            src = bass.AP(ptv.tensor, ptv.offset,
                          [[ptv.ap[0][0], 128], [15, 8], [1, 15]])
            nc.vector.tensor_copy(dst, src)
        wlinT.append(wt)

    def emit_phase1(b, oc):
        """weight-gen + softmax for one (b, oc); returns wn tile [t, (k h)]."""
        cs = b * C
        pw = ps_w.tile([TCH, J], F32)
        ptx = ps_t.tile([128, 8 * TCH], BF16, tag="ptt")
        for cc in range(8):
            nc.tensor.matmul(ptx[:, cc * TCH:(cc + 1) * TCH],
                             x_oc[oc][:, cs + cc * 128:cs + (cc + 1) * 128],
                             ident[:], is_transpose=True,
                             skip_group_check=True)
        xTw = xtp.tile([128, 8 * TCH], BF16, tag="xTw")
        nc.vector.tensor_copy(xTw[:], ptx[:])
        for cc in range(8):
            nc.tensor.matmul(pw[:], xTw[:, cc * TCH:(cc + 1) * TCH],
                             wlinT[cc][:], start=(cc == 0), stop=(cc == 7))
        # softmax over k (pw is [t, (k h)]: k stride 16, h stride 1)
        we = wex.tile([TCH, J], F32)
        nc.scalar.activation(we[:], pw[:], mybir.ActivationFunctionType.Exp)
        sums = wex.tile([TCH, H], F32, tag="sums")
        nc.vector.reduce_sum(sums[:],
                             we[:].rearrange("t (k h) -> t h k", h=H),
                             axis=mybir.AxisListType.X)
        inv = wex.tile([TCH, H], F32, tag="invs")
        nc.vector.reciprocal(inv[:], sums[:])
        wn = wnp.tile([TCH, J], BF16)
        nc.vector.tensor_tensor(
            wn[:].rearrange("t (k h) -> t k h", k=K),
            we[:].rearrange("t (k h) -> t k h", k=K),
            inv[:].unsqueeze(1).broadcast_to((TCH, K, H)),
            op=mybir.AluOpType.mult)
        return wn

    def emit_stage_write(b, oc, wn):
        at = adram[(b, oc)][:]
        dstw = bass.AP(at.tensor, at.offset, [[ROW_WR, TCH], [1, J]])
        nc.sync.dma_start(dstw, wn[:])

    def emit_readback(b, oc):
        at = adram[(b, oc)][:]
        apair = anat.tile([TCH, ROW_RD], BF16)
        srcr = bass.AP(at.tensor, at.offset, [[ROW_RD, TCH], [1, ROW_RD]])
        nc.sync.dma_start(apair[:], srcr)
        return apair

    def emit_phase3(b, oc, apair):
        """band transposes + conv + evac + store for one (b, oc)."""
        apv = apair[:]
        p0 = apv.ap[0][0]
        # halo bands: B[s<14, t] is nonzero only for t <= s < 14, so
        # transpose just the [14 t, 14 s] corner per h, packed as column
        # slices of one psum tile, single evac.
        pah = ps_a.tile([PAD_L, H * PAD_L], BF16, tag="pta")
        for h in range(H):
            src = bass.AP(apv.tensor, apv.offset + h,
                          [[p0, PAD_L], [H, PAD_L]])
            nc.tensor.matmul(pah[:, h * PAD_L:(h + 1) * PAD_L],
                             src, ident[0:PAD_L, 0:PAD_L],
                             is_transpose=True, skip_group_check=True)
        ba_c = bandp.tile([PAD_L, H * PAD_L], BF16, tag="ba")
        nc.vector.tensor_copy(ba_c[:], pah[:])
        # main bands: 16 transposes [128 t, 128 s] -> [128 s, 128 t]
        bwq_l = []
        for hq in range(2):
            pbq = ps_t.tile([128, 8 * TCH], BF16, tag="ptt")
            for j in range(8):
                h = hq * 8 + j
                src = bass.AP(apv.tensor, apv.offset + PAD_L * H + h,
                              [[p0, TCH], [H, TCH]])
                nc.tensor.matmul(pbq[:, j * TCH:(j + 1) * TCH],
                                 src, ident[:], is_transpose=True,
                                 skip_group_check=True)
            bwq = bandp.tile([128, 8 * TCH], BF16, tag="bw")
            if hq == 0:
                nc.vector.tensor_copy(bwq[:], pbq[:])
            else:
                nc.scalar.activation(bwq[:], pbq[:],
                                     mybir.ActivationFunctionType.Copy)
            bwq_l.append(bwq)
        # conv: out[t, r] = sum_s band[s, t] * x[s, b, h*64+r]
        stage = stg.tile([TCH, C], F32)
        xh_t = x_pre if oc == 0 else x_mid
        for hq in range(2):
            bwq = bwq_l[hq]
            pc = ps_c.tile([TCH, 8 * R], F32)
            for j in range(8):
                h = hq * 8 + j
                fo = b * C + h * R
                nc.tensor.matmul(pc[:, j * R:(j + 1) * R],
                                 bwq[:, j * TCH:(j + 1) * TCH],
                                 x_oc[oc][:, fo:fo + R],
                                 start=True, stop=False,
                                 skip_group_check=True)
                nc.tensor.matmul(pc[0:PAD_L, j * R:(j + 1) * R],
                                 ba_c[:, h * PAD_L:(h + 1) * PAD_L],
                                 xh_t[:, fo:fo + R],
                                 start=False, stop=True,
                                 skip_group_check=True)
            if hq == 0:
                nc.vector.tensor_copy(
                    stage[:, hq * 8 * R:(hq + 1) * 8 * R], pc[:])
            else:
                nc.scalar.activation(
                    stage[:, hq * 8 * R:(hq + 1) * 8 * R], pc[:],
                    mybir.ActivationFunctionType.Copy)
        nc.sync.dma_start(out_ap[oc * TCH:(oc + 1) * TCH, b, :], stage[:])

    # ---- software-pipelined unit loop: phase3 lags by SKEW units ------------
    units = [(b, oc) for b in range(B) for oc in range(OC)]
    ready = {}
    for i, (b, oc) in enumerate(units):
        wn = emit_phase1(b, oc)
        emit_stage_write(b, oc, wn)
        if i >= SKEW:
            pb, poc = units[i - SKEW]
            emit_phase3(pb, poc, ready.pop(i - SKEW))
        ready[i] = emit_readback(b, oc)
    for i in range(len(units) - SKEW, len(units)):
        pb, poc = units[i]
        emit_phase3(pb, poc, ready.pop(i))


def build_program(debug=False, reps=1):
    nc = bacc.Bacc("TRN2", target_bir_lowering=False, debug=debug,
                   enable_asserts=False, num_devices=N_CORES)
    x_t = nc.dram_tensor("x", [T_EXT, B, C], F32, kind="ExternalInput")
    wlin_t = nc.dram_tensor("wlin", [J, C], F32, kind="ExternalInput")
    out_t = nc.dram_tensor("out", [T_LOC, B, C], F32, kind="ExternalOutput")
    with tile.TileContext(nc) as tc:
        with tc.tile_pool(name="addram", bufs=1, space="DRAM") as apool, \
             tc.tile_pool(name="const", bufs=1) as const:
            adram = make_adram(apool)
            ident = const.tile([128, 128], BF16)
            ident_f = const.tile([120, 120], F32)
            if os.environ.get("NOPRELUDE") != "1":
                dynconv_prelude(tc, adram, ident, ident_f)
            if reps == 1:
                dynconv_kernel(tc, x_t.ap(), wlin_t.ap(), out_t.ap(),
                               adram, ident, ident_f)
            else:
                with tc.For_i(0, reps, 1):
                    dynconv_kernel(tc, x_t.ap(), wlin_t.ap(), out_t.ap(),
                                   adram, ident, ident_f)
    nc.compile()
    return nc


_NC_CACHE = None


def kernel(x: np.ndarray, Wlin: np.ndarray) -> np.ndarray:
    global _NC_CACHE
    if _NC_CACHE is None:
        _NC_CACHE = build_program()
    nc = _NC_CACHE
    xp = np.pad(x, ((PAD_L, 0), (0, 0), (0, 0)))
    in_maps = []
    for i in range(N_CORES):
        in_maps.append({
            "x": np.ascontiguousarray(xp[i * T_LOC:i * T_LOC + T_EXT]),
            "wlin": np.ascontiguousarray(Wlin),
        })
    res = run_bass_kernel_spmd(nc, in_maps, core_ids=list(range(N_CORES)))
    outs = [res.results[i]["out"] for i in range(N_CORES)]
    return np.concatenate(outs, axis=0)
